# revision 19
# baseline (speedup 1.0000x reference)
"""GATv2 2-layer + global-mean-pool classifier on 8 Trainium2 NeuronCores.

v2 strategy (device-time focused):
  - 50000 nodes sharded contiguously across 8 cores (6250 each, padded to
    6272 = 49 supertiles x 128).  Degree-sorted supertiles; edges padded to
    per-supertile max degree D_t; per-slot batched indirect DMA gathers.
  - bf16 tables / gathered rows / DVE elementwise path (fp32 reductions,
    exp, softmax denominators).  Rel tolerance is 2e-2; bf16 keeps ~1e-3.
  - NO layer-1 AllGather: every core receives the full node features
    (xT_full, bf16) and replicates the cheap phase-A table build
    (392 bf16 matmuls) into its own local DRAM copy of tbl1.
  - Layer-2 table AllGather is CHUNKED (7 chunks of 7 supertiles) and
    kicked as soon as the corresponding h1 tiles are done, overlapping the
    collective with the remaining layer-1 edge processing.  tbl2 rows use
    a chunk-major layout; idx2 is precomputed host-side to match.
  - e = att.LeakyReLU(xl_j+xr_i) via sign-split columns:
        e = 0.6*(attl_j+attr_i) + 0.4*(pos_abs - neg_abs)
    with |att|-prescaled table columns (attl in table cols 128/129).
  - Softmax division deferred past the PSUM-accumulated identity-matmul
    weighted sum; division via DVE tensor_scalar with per-partition ptr.
"""

import sys

import numpy as np

sys.path.insert(0, "/opt/trn_rl_repo")

# ---------------------------------------------------------------- constants
N = 50000
E = 600000
F_IN = 128
HID = 64
NG = 64
NCORES = 8
NSH_R = N // NCORES          # 6250 real nodes per core
NT = (NSH_R + 127) // 128    # 49 supertiles
NSH = NT * 128               # 6272 padded rank slots per core
TBL_N = NCORES * NSH         # 50176 table rows
MASK_NEG = -30.0
F1 = 132                     # L1 table row: 128 feats | attl(2) | pad(2)
F2 = 68                      # L2 table row: 64 feats | attl2(1) | pad(3)
CHUNK_T0 = [0, 13, 25, 37, 49]         # AG2 chunk supertile boundaries
NCHUNK = len(CHUNK_T0) - 1
CHUNK_R0 = [t * 128 for t in CHUNK_T0]             # row boundaries per core
CHUNK_ROWS = [CHUNK_R0[i + 1] - CHUNK_R0[i] for i in range(NCHUNK)]
CHUNK_BLK = np.concatenate([[0], np.cumsum([NCORES * r for r in CHUNK_ROWS])])


def _bf16(a):
    import jax.numpy as jnp
    return np.asarray(jnp.asarray(np.asarray(a, np.float32), jnp.bfloat16))


def _sign_split(att_row, W, scale_floor=1e-8):
    pos = np.where(att_row >= 0)[0]
    neg = np.where(att_row < 0)[0]
    perm = np.concatenate([pos, neg])
    scales = np.maximum(np.abs(att_row[perm]), scale_floor).astype(np.float32)
    Wsp = (W[:, perm] * scales[None, :]).astype(np.float32)
    return perm, len(pos), Wsp, scales


def prep(inputs):
    """All host-side restructuring. Returns (static, in_maps, host_ctx)."""
    x = np.asarray(inputs["x"], np.float32)
    ei = np.asarray(inputs["edge_index"], np.int64)
    batch = np.asarray(inputs["batch"], np.int64)
    Wl1 = np.asarray(inputs["Wl1"], np.float32)
    Wr1 = np.asarray(inputs["Wr1"], np.float32)
    att1 = np.asarray(inputs["att1"], np.float32)
    b1 = np.asarray(inputs["b1"], np.float32)
    Wl2 = np.asarray(inputs["Wl2"], np.float32)
    Wr2 = np.asarray(inputs["Wr2"], np.float32)
    att2 = np.asarray(inputs["att2"], np.float32)
    b2 = np.asarray(inputs["b2"], np.float32)

    loops = np.arange(N, dtype=np.int64)
    src = np.concatenate([ei[0], loops]).astype(np.int64)
    dst = np.concatenate([ei[1], loops]).astype(np.int64)

    # ---- per-head sign-split + scaling (layer 1) --------------------------
    P1 = np.zeros(2 * HID, np.int64)
    k1 = np.zeros(2, np.int64)
    Wl1s = np.zeros((F_IN, 2 * HID), np.float32)
    Wr1s = np.zeros((F_IN, 2 * HID), np.float32)
    inv1 = np.zeros(2 * HID, np.float32)
    for h in range(2):
        blk = slice(h * HID, (h + 1) * HID)
        perm, kp, Wsp, scales = _sign_split(att1[h], Wl1[:, blk])
        _, _, Wsp_r, _ = _sign_split(att1[h], Wr1[:, blk])
        P1[blk] = h * HID + perm
        k1[h] = kp
        Wl1s[:, blk] = Wsp
        Wr1s[:, blk] = Wsp_r
        inv1[blk] = 1.0 / scales
    wattl1 = np.stack([Wl1[:, h * HID:(h + 1) * HID] @ att1[h] for h in range(2)], 1)
    wattr1 = np.stack([Wr1[:, h * HID:(h + 1) * HID] @ att1[h] for h in range(2)], 1)
    Wlp1 = np.concatenate([Wl1s, wattl1, np.zeros((F_IN, 2), np.float32)], 1)
    Wrp1 = np.concatenate([Wr1s, wattr1, np.zeros((F_IN, 2), np.float32)], 1)

    # ---- layer 2 (heads=1); Wl2 rows permuted to device h1 order ----------
    Wl2d = Wl2[P1, :]
    Wr2d = Wr2[P1, :]
    P2, k2, Wl2s, scales2 = _sign_split(att2[0], Wl2d)
    _, _, Wr2s, _ = _sign_split(att2[0], Wr2d)
    inv2 = (1.0 / scales2).astype(np.float32)
    wattl2 = (Wl2d @ att2[0])[:, None]
    wattr2 = (Wr2d @ att2[0])[:, None]
    Wlp2 = np.concatenate([Wl2s, wattl2, np.zeros((2 * HID, 3), np.float32)], 1)
    Wrp2 = np.concatenate([Wr2s, wattr2, np.zeros((2 * HID, 3), np.float32)], 1)

    # ---- shard + degree-sort + supertile structure ------------------------
    core_of = dst // NSH_R
    order = np.argsort(core_of * np.int64(N) + dst, kind="stable")
    src_s, dst_s = src[order], dst[order]
    core_starts = np.searchsorted(core_of[order], np.arange(NCORES + 1))

    deg = np.bincount(dst, minlength=N)
    assert deg.max() <= 128, f"max degree {deg.max()} > 128"

    perm_nodes = np.zeros((NCORES, NSH), np.int64)
    rank_of = np.zeros(N, np.int64)
    Dt = np.zeros((NCORES, NT), np.int64)
    for c in range(NCORES):
        ids = np.arange(c * NSH_R, (c + 1) * NSH_R)
        p = np.argsort(-deg[ids], kind="stable")
        pn = ids[p]
        perm_nodes[c, :NSH_R] = pn
        perm_nodes[c, NSH_R:] = pn[0]
        rank_of[pn] = np.arange(NSH_R)
        dg = deg[pn].reshape(-1)
        for t in range(NT):
            lo, hi = t * 128, min((t + 1) * 128, NSH_R)
            Dt[c, t] = max(int(dg[lo:hi].max()) if hi > lo else 1, 1)
    D = np.maximum(Dt.max(0), 1)
    SD = int(D.sum())
    off = np.concatenate([[0], np.cumsum(D)]).astype(np.int64)

    core_of_n = np.arange(N) // NSH_R
    tbl_pos = core_of_n * NSH + rank_of                 # L1 table row of node
    # L2 chunk-major table row of node
    cc = np.searchsorted(CHUNK_R0, rank_of, side="right") - 1
    rows_c = np.asarray(CHUNK_ROWS)[cc]
    tbl2_pos = (CHUNK_BLK[cc] + core_of_n * rows_c
                + (rank_of - np.asarray(CHUNK_R0)[cc]))

    # ---- per-core gather idx + mask ---------------------------------------
    idx1_h = np.zeros((NCORES, 128, SD), np.int32)
    idx2_h = np.zeros((NCORES, 128, SD), np.int32)
    mask_h = np.full((NCORES, 128, SD), MASK_NEG, np.float32)
    batch_h = np.full((NCORES, 128, NT), -1.0, np.float32)
    xTs_h = np.zeros((NCORES, F_IN, NSH), np.float32)
    for c in range(NCORES):
        e0, e1 = core_starts[c], core_starts[c + 1]
        s_c, d_c = src_s[e0:e1], dst_s[e0:e1]
        r_c = rank_of[d_c]
        eo = np.argsort(r_c, kind="stable")
        s_c, r_c = s_c[eo], r_c[eo]
        starts = np.searchsorted(r_c, np.arange(NSH + 1))
        slot = np.arange(len(r_c)) - starts[r_c]
        t_of = r_c // 128
        p_of = r_c % 128
        idx1_h[c, p_of, off[t_of] + slot] = tbl_pos[s_c].astype(np.int32)
        idx2_h[c, p_of, off[t_of] + slot] = tbl2_pos[s_c].astype(np.int32)
        mask_h[c, p_of, off[t_of] + slot] = 0.0
        rb = np.full(NSH, -1.0, np.float32)
        rb[:NSH_R] = batch[perm_nodes[c, :NSH_R]].astype(np.float32)
        batch_h[c] = rb.reshape(NT, 128).T
        xTs_h[c] = x[perm_nodes[c]].T
        xTs_h[c, :, NSH_R:] = 0.0

    # full node-feature matrix in table-row order (all cores identical)
    xT_full = x[perm_nodes.reshape(-1)].T.astype(np.float32)   # [128, TBL_N]

    const_row = lambda v: np.tile(np.asarray(v, np.float32)[None, :], (128, 1))
    static = dict(D=[int(d) for d in D], SD=SD,
                  k1=[int(v) for v in k1], k2=int(k2))
    common = {
        "xTfull": _bf16(xT_full),
        "wlp1": _bf16(Wlp1), "wrp1": _bf16(Wrp1),
        "wlp2": _bf16(Wlp2), "wrp2": _bf16(Wrp2),
        "attinv1": const_row(inv1), "b1t": const_row(b1[P1]),
        "attinv2": const_row(inv2)[:, :64], "b2t": const_row(b2[P2])[:, :64],
        "iota64": np.tile(np.arange(64, dtype=np.float32)[None, :], (128, 1)),
        "ident": _bf16(np.eye(128, dtype=np.float32)),
    }
    in_maps = []
    for c in range(NCORES):
        m = dict(common)
        m["xTs"] = _bf16(xTs_h[c])
        m["idx1"] = idx1_h[c]
        m["idx2"] = idx2_h[c]
        m["maskt"] = mask_h[c]
        m["batchv"] = batch_h[c]
        in_maps.append(m)

    host_ctx = dict(
        batch=batch, P2=P2, k1=[int(v) for v in k1], k2=int(k2),
        Wlin=np.asarray(inputs["Wlin"], np.float32),
        blin=np.asarray(inputs["blin"], np.float32),
    )
    return static, in_maps, host_ctx


def host_epilogue(partials, host_ctx):
    pooled = np.sum(np.stack(partials, 0), 0)
    counts = np.bincount(host_ctx["batch"], minlength=NG).astype(np.float32)
    g = pooled / np.maximum(counts, 1.0)[:, None]
    Wlin_p = host_ctx["Wlin"][host_ctx["P2"], :]
    return (g @ Wlin_p + host_ctx["blin"]).astype(np.float32)


# ---------------------------------------------------------------- numpy mock
def numpy_device_mock(static, in_maps, host_ctx):
    """fp32 functional mock of the v2 device kernel (bf16 rounding applied to
    tables / gathered rows to estimate precision)."""
    D, SD = static["D"], static["SD"]
    off = np.concatenate([[0], np.cumsum(D)]).astype(np.int64)
    k1, k2 = static["k1"], static["k2"]
    bf = lambda a: _bf16(a).astype(np.float32)

    m0 = in_maps[0]
    xT_full = np.asarray(m0["xTfull"], np.float32)
    wlp1 = np.asarray(m0["wlp1"], np.float32)
    wrp1 = np.asarray(m0["wrp1"], np.float32)
    wlp2 = np.asarray(m0["wlp2"], np.float32)
    wrp2 = np.asarray(m0["wrp2"], np.float32)

    # replicated phase A (same on every core)
    tbl1 = bf(xT_full.T @ wlp1)                        # [TBL_N, F1]

    partials = []
    tbl2 = np.zeros((TBL_N, F2), np.float32)
    h1_all = {}
    for c, m in enumerate(in_maps):
        xTs = np.asarray(m["xTs"], np.float32)
        xre1 = bf(xTs.T @ wrp1)                        # [NSH, F1]
        h1c = np.zeros((NSH, 128), np.float32)
        for t in range(NT):
            d = D[t]
            idx = np.asarray(m["idx1"])[:, off[t]:off[t] + d]
            msk = np.asarray(m["maskt"])[:, off[t]:off[t] + d]
            A = tbl1[idx.reshape(-1)].reshape(128, d, F1)
            xr = xre1[t * 128:(t + 1) * 128]           # [128, F1]
            s = bf(A[:, :, :128] + xr[:, None, :128])
            e = np.zeros((128, 2, d), np.float32)
            for h in range(2):
                base = h * 64
                pos = np.abs(s[:, :, base:base + k1[h]]).sum(2)
                neg = np.abs(s[:, :, base + k1[h]:base + 64]).sum(2)
                attl = A[:, :, 128 + h]
                attr = xr[:, 128 + h]
                e[:, h] = 0.6 * (attl + attr[:, None]) + 0.4 * (pos - neg) + msk
            p = np.exp(e)
            denom = p.sum(2)
            W = np.concatenate([
                bf(A[:, :, 0:64] * p[:, 0, :, None]),
                bf(A[:, :, 64:128] * p[:, 1, :, None])], 2)
            outw = W.sum(1)
            hh = np.concatenate([outw[:, 0:64] / denom[:, 0:1],
                                 outw[:, 64:128] / denom[:, 1:2]], 1)
            hh = hh * np.asarray(m["attinv1"]) + np.asarray(m["b1t"])
            hh = np.maximum(hh, np.exp(np.minimum(hh, 0.0)) - 1.0)
            hh = bf(hh)
            h1c[t * 128:(t + 1) * 128] = hh
            r0 = t * 128
            cc = int(np.searchsorted(CHUNK_R0, r0, side="right")) - 1
            base = (CHUNK_BLK[cc] + c * CHUNK_ROWS[cc] + (r0 - CHUNK_R0[cc]))
            tbl2[base:base + 128] = bf(hh @ wlp2)
        h1_all[c] = h1c

    for c, m in enumerate(in_maps):
        h1c = h1_all[c]
        xre2 = bf(h1c @ wrp2)                          # [NSH, F2]
        pooled = np.zeros((64, 64), np.float32)
        for t in range(NT):
            d = D[t]
            idx = np.asarray(m["idx2"])[:, off[t]:off[t] + d]
            msk = np.asarray(m["maskt"])[:, off[t]:off[t] + d]
            A = tbl2[idx.reshape(-1)].reshape(128, d, F2)
            xr = xre2[t * 128:(t + 1) * 128]
            s = bf(A[:, :, :64] + xr[:, None, :64])
            pos = np.abs(s[:, :, 0:k2]).sum(2)
            neg = np.abs(s[:, :, k2:64]).sum(2)
            e = 0.6 * (A[:, :, 64] + xr[:, 64][:, None]) + 0.4 * (pos - neg) + msk
            p = np.exp(e)
            denom = p.sum(1)
            outw = bf(A[:, :, 0:64] * p[:, :, None]).sum(1)
            hh = outw / denom[:, None] * np.asarray(m["attinv2"]) \
                + np.asarray(m["b2t"])
            hh = np.maximum(hh, np.exp(np.minimum(hh, 0.0)) - 1.0)
            hh = bf(hh)
            onehot = (np.asarray(m["iota64"]) ==
                      np.asarray(m["batchv"])[:, t:t + 1]).astype(np.float32)
            pooled += onehot.T @ hh
        partials.append(pooled)
    return host_epilogue(partials, host_ctx)


# ---------------------------------------------------------------- device impl
def build_nc(static):
    import concourse.bass as bass
    import concourse.bacc as bacc
    import concourse.mybir as mybir
    import concourse.tile as tile
    from contextlib import ExitStack

    fp32 = mybir.dt.float32
    bf16 = mybir.dt.bfloat16
    i32 = mybir.dt.int32
    AF = mybir.ActivationFunctionType
    OP = mybir.AluOpType

    D, SD = static["D"], static["SD"]
    off = np.concatenate([[0], np.cumsum(D)]).astype(np.int64)
    k1, k2 = static["k1"], static["k2"]
    maxD = max(D)

    nc = bacc.Bacc(None, num_devices=NCORES)

    # ---- I/O ----
    xTfull = nc.dram_tensor("xTfull", [F_IN, TBL_N], bf16, kind="ExternalInput")
    xTs = nc.dram_tensor("xTs", [F_IN, NSH], bf16, kind="ExternalInput")
    wlp1 = nc.dram_tensor("wlp1", [F_IN, F1], bf16, kind="ExternalInput")
    wrp1 = nc.dram_tensor("wrp1", [F_IN, F1], bf16, kind="ExternalInput")
    wlp2 = nc.dram_tensor("wlp2", [2 * HID, F2], bf16, kind="ExternalInput")
    wrp2 = nc.dram_tensor("wrp2", [2 * HID, F2], bf16, kind="ExternalInput")
    idx1 = nc.dram_tensor("idx1", [128, SD], i32, kind="ExternalInput")
    idx2 = nc.dram_tensor("idx2", [128, SD], i32, kind="ExternalInput")
    maskt = nc.dram_tensor("maskt", [128, SD], fp32, kind="ExternalInput")
    batchv = nc.dram_tensor("batchv", [128, NT], fp32, kind="ExternalInput")
    attinv1 = nc.dram_tensor("attinv1", [128, 128], fp32, kind="ExternalInput")
    b1t = nc.dram_tensor("b1t", [128, 128], fp32, kind="ExternalInput")
    attinv2 = nc.dram_tensor("attinv2", [128, 64], fp32, kind="ExternalInput")
    b2t = nc.dram_tensor("b2t", [128, 64], fp32, kind="ExternalInput")
    iota64 = nc.dram_tensor("iota64", [128, 64], fp32, kind="ExternalInput")
    ident = nc.dram_tensor("ident", [128, 128], bf16, kind="ExternalInput")
    pooled_out = nc.dram_tensor("pooled", [64, 64], fp32, kind="ExternalOutput")

    tbl1_full = nc.dram_tensor("tbl1_full", [TBL_N, F1], bf16)   # local, replicated
    tbl2_sh = nc.dram_tensor("tbl2_sh", [NSH, F2], bf16)
    tbl2_full = nc.dram_tensor("tbl2_full", [TBL_N, F2], bf16, addr_space="Shared")

    with tile.TileContext(nc) as tc, ExitStack() as ctx:
        cp = ctx.enter_context(tc.tile_pool(name="const", bufs=1))
        wlp1_s = cp.tile([F_IN, F1], bf16); nc.scalar.dma_start(wlp1_s[:], wlp1[:, :])
        wrp1_s = cp.tile([F_IN, F1], bf16); nc.scalar.dma_start(wrp1_s[:], wrp1[:, :])
        wlp2_s = cp.tile([2 * HID, F2], bf16); nc.scalar.dma_start(wlp2_s[:], wlp2[:, :])
        wrp2_s = cp.tile([2 * HID, F2], bf16); nc.scalar.dma_start(wrp2_s[:], wrp2[:, :])
        idx1_s = cp.tile([128, SD], i32); nc.gpsimd.dma_start(idx1_s[:], idx1[:, :])
        idx2_s = cp.tile([128, SD], i32); nc.gpsimd.dma_start(idx2_s[:], idx2[:, :])
        mask_s = cp.tile([128, SD], fp32); nc.gpsimd.dma_start(mask_s[:], maskt[:, :])
        batch_s = cp.tile([128, NT], fp32); nc.scalar.dma_start(batch_s[:], batchv[:, :])
        ai1_s = cp.tile([128, 128], fp32); nc.scalar.dma_start(ai1_s[:], attinv1[:, :])
        b1_s = cp.tile([128, 128], fp32); nc.scalar.dma_start(b1_s[:], b1t[:, :])
        ai2_s = cp.tile([128, 64], fp32); nc.scalar.dma_start(ai2_s[:], attinv2[:, :])
        b2_s = cp.tile([128, 64], fp32); nc.scalar.dma_start(b2_s[:], b2t[:, :])
        io64_s = cp.tile([128, 64], fp32); nc.scalar.dma_start(io64_s[:], iota64[:, :])
        id_s = cp.tile([128, 128], bf16); nc.scalar.dma_start(id_s[:], ident[:, :])

        big = ctx.enter_context(tc.tile_pool(name="big", bufs=1))
        xre1_s = big.tile([128, NT * F1], bf16)
        xre2_s = big.tile([128, NT * F2], bf16)
        h1_s = big.tile([128, NT * 128], bf16)

        GBATCH = 4                        # table tiles per staged DMA store
        # ---------------- phase A: replicated layer-1 table ----------------
        with tc.tile_pool(name="phA_x", bufs=2) as pxt, \
             tc.tile_pool(name="phA", bufs=3) as pa, \
             tc.tile_pool(name="phA_ps", bufs=4, space="PSUM") as pap:
            # xre1 for own shard first (small, frees xTs quickly)
            xTs_s = pxt.tile([F_IN, NSH], bf16, tag="xTs")
            nc.sync.dma_start(xTs_s[:], xTs[:, :])
            for t in range(NT):
                ps = pap.tile([128, F1], fp32, tag="psA")
                nc.tensor.matmul(ps[:], xTs_s[:, t * 128:(t + 1) * 128],
                                 wrp1_s[:], start=True, stop=True)
                nc.vector.tensor_scalar(xre1_s[:, t * F1:(t + 1) * F1], ps[:],
                                        0.0, None, op0=OP.add)
            # full table, loaded+computed in 8 column chunks; DMA queues
            # round-robin over SP / ACT / Pool (Pool is idle in phase A),
            # PSUM->SBUF copies paired on DVE
            ndma = 0
            for ch in range(NCORES):
                xf = pxt.tile([F_IN, NSH], bf16, tag="xf")
                eng = (nc.sync, nc.scalar, nc.gpsimd)[ndma % 3]; ndma += 1
                eng.dma_start(xf[:], xTfull[:, ch * NSH:(ch + 1) * NSH])
                for gb in range(NT // GBATCH + 1):
                    g0 = gb * GBATCH
                    gn = min(GBATCH, NT - g0)
                    if gn <= 0:
                        break
                    stg = pa.tile([128, GBATCH * F1], bf16, tag="stgA")
                    for gg in range(gn):
                        g = g0 + gg
                        ps = pap.tile([128, F1], fp32, tag="psT")
                        nc.tensor.matmul(ps[:], xf[:, g * 128:(g + 1) * 128],
                                         wlp1_s[:], start=True, stop=True)
                        if gg % 2 == 0:
                            nc.vector.tensor_scalar(
                                stg[:, gg * F1:(gg + 1) * F1], ps[:],
                                0.0, None, op0=OP.add)
                        else:
                            nc.scalar.copy(stg[:, gg * F1:(gg + 1) * F1], ps[:])
                    # store [128, gn*F1] -> tbl1 rows [base, base+gn*128)
                    # (partition p of stg block gg -> row base+gg*128+p)
                    base = ch * NSH + g0 * 128
                    tref = tbl1_full[:, :]
                    dst = bass.AP(tref.tensor, base * F1,
                                  [[F1, 128], [128 * F1, gn], [1, F1]])
                    eng = (nc.sync, nc.scalar, nc.gpsimd)[ndma % 3]; ndma += 1
                    eng.dma_start(dst, stg[:, :gn * F1])

        # ---------------- edge layer ----------------
        def edge_layer(tblT, xre_s, Fw, nheads, kpos, ai_s, bt_s, h_w, idx_s,
                       per_tile_done, h_out=None):
            nch = 64  # feature channels per head
            with tc.tile_pool(name=f"edg{Fw}", bufs=3) as pe, \
                 tc.tile_pool(name=f"sm{Fw}", bufs=3) as psm, \
                 tc.tile_pool(name=f"ps{Fw}", bufs=2, space="PSUM") as pps:
                for t in range(NT):
                    d = D[t]
                    A = pe.tile([128, maxD * Fw], bf16, tag="A")
                    for kk in range(d):
                        nc.gpsimd.indirect_dma_start(
                            out=A[:, kk * Fw:(kk + 1) * Fw],
                            out_offset=None,
                            in_=tblT[:, :],
                            in_offset=bass.IndirectOffsetOnAxis(
                                ap=idx_s[:, int(off[t]) + kk:int(off[t]) + kk + 1],
                                axis=0),
                        )
                    A3 = A[:, :d * Fw].rearrange("p (d f) -> p d f", f=Fw)
                    xr = xre_s[:, t * Fw:(t + 1) * Fw]
                    # s = A + xr over feature channels only (bf16)
                    nf = nheads * nch
                    s = pe.tile([128, maxD * 128], bf16, tag="s")
                    s3 = s[:, :d * nf].rearrange("p (d f) -> p d f", f=nf)
                    xrb = bass.AP(xr.tensor, xr.offset,
                                  [xr.ap[0], [0, d], [1, nf]])
                    nc.vector.tensor_tensor(s3, A3[:, :, :nf], xrb, op=OP.add)
                    # abs-reduce pos/neg blocks per head -> fp32
                    ew = psm.tile([128, 4 * maxD], fp32, tag="ew")
                    for h in range(nheads):
                        base = h * nch
                        nc.vector.tensor_reduce(
                            ew[:, (2 * h) * d:(2 * h) * d + d],
                            s3[:, :, base:base + kpos[h]],
                            axis=mybir.AxisListType.X, op=OP.add,
                            apply_absolute_value=True)
                        nc.vector.tensor_reduce(
                            ew[:, (2 * h + 1) * d:(2 * h + 1) * d + d],
                            s3[:, :, base + kpos[h]:base + nch],
                            axis=mybir.AxisListType.X, op=OP.add,
                            apply_absolute_value=True)
                    pn = psm.tile([128, 2 * maxD], fp32, tag="pn")
                    ew4 = ew[:, :4 * d].rearrange("p (s d) -> p s d", d=d)
                    pnv = pn[:, :nheads * d].rearrange("p (s d) -> p s d", d=d)
                    nc.vector.tensor_tensor(
                        pnv, ew4[:, 0:2 * nheads:2, :], ew4[:, 1:2 * nheads:2, :],
                        op=OP.subtract)
                    # 0.6*attr + mask
                    am = psm.tile([128, 2 * maxD], fp32, tag="am")
                    amv = am[:, :nheads * d].rearrange("p (s d) -> p s d", d=d)
                    mvec = mask_s[:, int(off[t]):int(off[t]) + d]
                    mb = bass.AP(mvec.tensor, mvec.offset,
                                 [mvec.ap[0], [0, nheads], [1, d]])
                    attr = xr[:, nf:nf + nheads]
                    attrb = bass.AP(attr.tensor, attr.offset,
                                    [attr.ap[0], [1, nheads], [0, d]])
                    tmp = psm.tile([128, 2 * maxD], fp32, tag="amt")
                    tmpv = tmp[:, :nheads * d].rearrange("p (s d) -> p s d", d=d)
                    nc.vector.scalar_tensor_tensor(
                        tmpv, attrb, 0.6, mb, op0=OP.mult, op1=OP.add)
                    nc.vector.scalar_tensor_tensor(
                        amv, pnv, 0.4, tmpv, op0=OP.mult, op1=OP.add)
                    attlv = bass.AP(A.tensor, A.offset + nf,
                                    [A.ap[0], [1, nheads], [Fw, d]])
                    ee = psm.tile([128, 2 * maxD], fp32, tag="ee")
                    eev = ee[:, :nheads * d].rearrange("p (s d) -> p s d", d=d)
                    nc.vector.scalar_tensor_tensor(
                        eev, attlv, 0.6, amv, op0=OP.mult, op1=OP.add)
                    pexp = psm.tile([128, 2 * maxD], fp32, tag="pexp")
                    nc.scalar.activation(pexp[:, :nheads * d],
                                         ee[:, :nheads * d], AF.Exp)
                    pexp3 = pexp[:, :nheads * d].rearrange("p (s d) -> p s d", d=d)
                    den = psm.tile([128, 2], fp32, tag="den")
                    nc.vector.tensor_reduce(den[:, :nheads], pexp3,
                                            axis=mybir.AxisListType.X, op=OP.add)
                    rd = psm.tile([128, 2], fp32, tag="rd")
                    nc.vector.reciprocal(rd[:, :nheads], den[:, :nheads])
                    # W = A * exp (per head, bf16)
                    W = pe.tile([128, maxD * h_w], bf16, tag="W")
                    W3 = W[:, :d * h_w].rearrange("p (d f) -> p d f", f=h_w)
                    for h in range(nheads):
                        eb = bass.AP(pexp.tensor, pexp.offset + h * d,
                                     [pexp.ap[0], [1, d], [0, nch]])
                        nc.vector.tensor_tensor(
                            W3[:, :, h * nch:(h + 1) * nch],
                            A3[:, :, h * nch:(h + 1) * nch], eb, op=OP.mult)
                    po = pps.tile([128, h_w], fp32, tag="po")
                    for dd in range(d):
                        nc.tensor.matmul(po[:], id_s[:], W3[:, dd, :],
                                         start=(dd == 0), stop=(dd == d - 1))
                    # epilogue: /denom, descale, +bias, elu -> bf16
                    hh = psm.tile([128, h_w], fp32, tag="hh")
                    for h in range(nheads):
                        nc.vector.tensor_scalar(
                            hh[:, h * nch:(h + 1) * nch],
                            po[:, h * nch:(h + 1) * nch],
                            rd[:, h:h + 1], None, op0=OP.mult)
                    nc.vector.tensor_tensor(hh[:], hh[:], ai_s[:, :h_w], op=OP.mult)
                    nc.vector.tensor_tensor(hh[:], hh[:], bt_s[:, :h_w], op=OP.add)
                    mn = psm.tile([128, h_w], fp32, tag="mn")
                    nc.vector.tensor_scalar(mn[:], hh[:], 0.0, None, op0=OP.min)
                    ex = psm.tile([128, h_w], fp32, tag="ex")
                    nc.scalar.activation(ex[:], mn[:], AF.Exp)
                    if h_out is not None:
                        hv = h_out[:, t * h_w:(t + 1) * h_w]
                    else:
                        hbf = psm.tile([128, h_w], bf16, tag="hbf")
                        hv = hbf[:]
                    nc.vector.scalar_tensor_tensor(
                        hv, ex[:], -1.0, hh[:], op0=OP.add, op1=OP.max)
                    per_tile_done(t, hv, pe, pps)

        # ---------------- phase B: layer-1 edges + interleaved tbl2 --------
        collective_chunks = []

        def l1_tile_done(t, hv, pe, pps):
            # transpose h1 tile, compute tbl2 + xre2 rows, store shard
            psT = pps.tile([128, 128], bf16, tag="psTr")
            nc.tensor.transpose(psT[:], hv, id_s[:])
            h1T = pe.tile([128, 128], bf16, tag="h1T")
            nc.scalar.copy(h1T[:], psT[:])
            ps2 = pps.tile([128, F2], fp32, tag="psC")
            nc.tensor.matmul(ps2[:], h1T[:], wlp2_s[:], start=True, stop=True)
            stg = pe.tile([128, F2], bf16, tag="stgC")
            nc.scalar.copy(stg[:], ps2[:])
            nc.sync.dma_start(tbl2_sh[t * 128:(t + 1) * 128, :], stg[:])
            ps3 = pps.tile([128, F2], fp32, tag="psC")
            nc.tensor.matmul(ps3[:], h1T[:], wrp2_s[:], start=True, stop=True)
            nc.scalar.copy(xre2_s[:, t * F2:(t + 1) * F2], ps3[:])
            if (t + 1) in CHUNK_T0[1:]:
                c = CHUNK_T0.index(t + 1) - 1
                collective_chunks.append(c)
                nc.gpsimd.collective_compute(
                    "AllGather", mybir.AluOpType.bypass,
                    replica_groups=[list(range(NCORES))],
                    ins=[tbl2_sh[CHUNK_R0[c]:CHUNK_R0[c + 1], :]],
                    outs=[tbl2_full[int(CHUNK_BLK[c]):int(CHUNK_BLK[c + 1]), :]],
                )

        edge_layer(tbl1_full, xre1_s, F1, 2, k1, ai1_s, b1_s, 128, idx1_s,
                   l1_tile_done, h_out=h1_s)
        assert len(collective_chunks) == NCHUNK

        # ---------------- phase D: layer-2 edges + pooling -----------------
        with tc.tile_pool(name="phE_ps", bufs=1, space="PSUM") as pep:
            psP = pep.tile([64, 64], fp32)

            def l2_tile_done(t, hv, pe, pps):
                oh = pe.tile([128, 64], bf16, tag="oh")
                nc.vector.tensor_scalar(oh[:], io64_s[:], batch_s[:, t:t + 1],
                                        None, op0=OP.is_equal)
                nc.tensor.matmul(psP[:], oh[:], hv,
                                 start=(t == 0), stop=(t == NT - 1))

            edge_layer(tbl2_full, xre2_s, F2, 1, [k2], ai2_s, b2_s, 64, idx2_s,
                       l2_tile_done)
            with tc.tile_pool(name="phE", bufs=1) as pe_:
                stg = pe_.tile([64, 64], fp32, tag="stgE")
                nc.scalar.copy(stg[:], psP[:])
                nc.sync.dma_start(pooled_out[:, :], stg[:])

    nc.finalize()
    return nc


_CACHE = {}


def kernel(**inputs) -> np.ndarray:
    static, in_maps, host_ctx = prep(inputs)
    key = (tuple(static["D"]), tuple(static["k1"]), static["k2"])
    if key not in _CACHE:
        _CACHE[key] = build_nc(static)
    nc = _CACHE[key]
    from concourse.bass_utils import run_bass_kernel_spmd
    res = run_bass_kernel_spmd(nc, in_maps, core_ids=list(range(NCORES)))
    partials = [r["pooled"] for r in res.results]
    return host_epilogue(partials, host_ctx)
